# revision 7
# baseline (speedup 1.0000x reference)
"""CompressedLinear TRN2 kernel: y = x @ ((w_q - zp) * scale).T + bias

Shapes (hardcoded): x [4,2048,4096] f32, weight_q [4096,4096] i32 (values 0..255),
weight_zero_point [4096] i32, weight_scale [4096] f32, bias [4096] f32.

Sharding: column-parallel over 8 NeuronCores (per the tensor-parallel hint).
Core c owns output features [c*512, (c+1)*512): it receives the full
activations and its 512-row slice of the quantized weight (+zp/scale/bias).

Design (measured at ~460 us/core steady-state vs the 437 us PE roofline):
- Activations are pre-tiled on host into [slab, partition(k), k_outer, m]
  so each DMA slab is one fully-contiguous 32KB run per partition
  (strided layouts measured ~240 GB/s; this layout keeps DMA hidden).
- Matmuls run as float32r (fp32 storage, reduced-precision multiply):
  measured same PE rate as fp16 (1 moving column/cycle) with better
  accuracy (rel err 1.7e-4 vs 2.3e-4), and no host-side downcast of x.
- Weights are dequantized on-device: (w_q - zp) * scale -> float32r,
  one [128, 512] tile per k-slice so the matmul stream starts as soon as
  the first k-tile is ready. Weights stay SBUF-resident (8.4MB).
- PSUM accumulates fp32 over the 32 k-tiles per [128m x 512o] tile;
  epilogue adds bias during the PSUM->SBUF copy on the vector engine;
  outputs stream back on the scalar engine's DMA ring.
"""

import numpy as np

B, S, IN, OUT = 4, 2048, 4096, 4096
M = B * S  # 8192 tokens
NCORES = 8
OSH = OUT // NCORES  # 512 output features per core
P = 128
KO = IN // P  # 32 k-tiles
MT = 256  # tokens per streamed activation slab
N_SLABS = M // MT  # 32
MSUB = MT // P  # 2 psum groups per slab


def _split_waits(nc, mybir, max_waits=1):
    """walrus in this env rejects >1 sem wait on drain/self-loading-matmul
    instructions; hoist extra waits onto same-engine NoOps just before."""
    for bb in nc.m.functions[0].blocks:
        new_list = []
        for inst in bb.instructions:
            si = inst.sync_info
            if si and si.on_wait and len(si.on_wait) > max_waits:
                waits = list(si.on_wait)
                extra, keep = waits[max_waits:], waits[:max_waits]
                for j, w in enumerate(extra):
                    nop = mybir.InstNoOp(name=f"{inst.name}-waitsplit-{j}", ins=[], outs=[])
                    nop.engine = inst.engine
                    nop.sync_info = mybir.SyncInfo(on_wait=[w], on_update=[])
                    nc.register_instruction(nop)
                    new_list.append(nop)
                inst.sync_info = mybir.SyncInfo(on_wait=keep, on_update=list(si.on_update))
            new_list.append(inst)
        bb.instructions = new_list


def build_module(repeat=1):
    import concourse.bass as bass
    import concourse.tile as tile
    import concourse.mybir as mybir

    nc = bass.Bass(trn_type="TRN2", target_bir_lowering=False, debug=False)
    f32 = mybir.dt.float32
    f32r = mybir.dt.float32r
    i32 = mybir.dt.int32

    xt = nc.dram_tensor("xt", [N_SLABS, P, KO, MT], f32r, kind="ExternalInput").ap()
    wtq = nc.dram_tensor("wtq", [IN, OSH], i32, kind="ExternalInput").ap()
    zp = nc.dram_tensor("zp", [OSH], i32, kind="ExternalInput").ap()
    scale = nc.dram_tensor("scale", [OSH], f32, kind="ExternalInput").ap()
    bias = nc.dram_tensor("bias", [OSH], f32, kind="ExternalInput").ap()
    y = nc.dram_tensor("y", [M, OSH], f32, kind="ExternalOutput").ap()

    wtq_r = wtq.rearrange("(ko p) o -> p ko o", p=P)  # [128, 32, 512]

    with tile.TileContext(nc) as tc:
        with (
            tc.tile_pool(name="wpool", bufs=1) as wpool,
            tc.tile_pool(name="cpool", bufs=1) as cpool,
            tc.tile_pool(name="spool", bufs=3) as spool,
            tc.tile_pool(name="xpool", bufs=3) as xpool,
            tc.tile_pool(name="opool", bufs=4) as opool,
            tc.tile_pool(name="ppool", bufs=8, space="PSUM") as ppool,
        ):
            # --- constants (broadcast along partitions via step-0 DMA) ---
            zp_b = cpool.tile([P, OSH], i32, tag="zp_b")
            nc.sync.dma_start(zp_b[:], zp.partition_broadcast(P))
            scale_b = cpool.tile([P, OSH], f32, tag="scale_b")
            nc.sync.dma_start(scale_b[:], scale.partition_broadcast(P))
            bias_b = cpool.tile([P, OSH], f32, tag="bias_b")
            nc.sync.dma_start(bias_b[:], bias.partition_broadcast(P))

            # --- dequantize weights into 32 resident SBUF tiles [128, 512] ---
            wt_l = []
            for ko in range(KO):
                stage = spool.tile([P, OSH], i32, tag="stage")
                # scalar ring: keeps the sync ring free for activation slabs
                nc.scalar.dma_start(stage[:], wtq_r[:, ko, :])
                tmp = spool.tile([P, OSH], f32, tag="tmp")
                nc.vector.tensor_tensor(tmp[:], stage[:], zp_b[:], mybir.AluOpType.subtract)
                wt = wpool.tile([P, OSH], f32r, tag=f"wt{ko}")
                nc.vector.tensor_tensor(wt[:], tmp[:], scale_b[:], mybir.AluOpType.mult)
                wt_l.append(wt)

            # --- stream activations, matmul, epilogue ---
            for _ in range(repeat):
                for sl in range(N_SLABS):
                    x_sb = xpool.tile([P, KO, MT], f32r, tag="x_sb")
                    nc.sync.dma_start(x_sb[:], xt[sl])
                    for ms in range(MSUB):
                        psum = ppool.tile([P, OSH], f32, tag="psum")
                        for ko in range(KO):
                            nc.tensor.matmul(
                                psum[:],
                                x_sb[:, ko, ms * P : (ms + 1) * P],
                                wt_l[ko][:],
                                start=(ko == 0),
                                stop=(ko == KO - 1),
                            )
                        out_sb = opool.tile([P, OSH], f32, tag="out_sb")
                        nc.vector.tensor_tensor(
                            out_sb[:], psum[:], bias_b[:], mybir.AluOpType.add
                        )
                        m0 = sl * MT + ms * P
                        nc.scalar.dma_start(y[m0 : m0 + P, :], out_sb[:])

    _split_waits(nc, mybir)
    return nc


def shard_inputs(x, weight_q, weight_zero_point, weight_scale, bias):
    # tiled layout: xt[sl, p, ko, m] = x[sl*MT + m, ko*P + p]
    xt = np.ascontiguousarray(
        x.reshape(N_SLABS, MT, KO, P).transpose(0, 3, 2, 1).astype(np.float32)
    )
    in_maps = []
    for c in range(NCORES):
        sl = slice(c * OSH, (c + 1) * OSH)
        in_maps.append(
            {
                "xt": xt,
                "wtq": np.ascontiguousarray(weight_q[sl, :].T),  # [4096, 512] i32
                "zp": np.ascontiguousarray(weight_zero_point[sl]),
                "scale": np.ascontiguousarray(weight_scale[sl]),
                "bias": np.ascontiguousarray(bias[sl]),
            }
        )
    return in_maps


def kernel(x, weight_q, weight_zero_point, weight_scale, bias):
    from concourse.bass_utils import run_bass_kernel_spmd

    x = np.asarray(x, dtype=np.float32)
    weight_q = np.asarray(weight_q, dtype=np.int32)
    weight_zero_point = np.asarray(weight_zero_point, dtype=np.int32)
    weight_scale = np.asarray(weight_scale, dtype=np.float32)
    bias = np.asarray(bias, dtype=np.float32)

    nc = build_module()
    in_maps = shard_inputs(x, weight_q, weight_zero_point, weight_scale, bias)
    try:
        res = run_bass_kernel_spmd(nc, in_maps, core_ids=list(range(NCORES)), trace=False)
    except Exception:
        # transient device wedges (NRT_EXEC_UNIT_UNRECOVERABLE) have been
        # observed to clear on retry; on native NRT a core reset helps too
        import os as _os
        import time as _time

        _os.environ.setdefault("NEURON_RT_RESET_CORES", "1")
        _time.sleep(5)
        res = run_bass_kernel_spmd(nc, in_maps, core_ids=list(range(NCORES)), trace=False)
    shards = [res.results[c]["y"] for c in range(NCORES)]  # each [8192, 512]
    return np.concatenate(shards, axis=1).reshape(B, S, OUT)
